# revision 12
# baseline (speedup 1.0000x reference)
"""Criss-cross attention block (CCNet) Bass/Tile kernel for Trainium2.

Shapes (hardcoded): B=8, C=256, H=W=128, CR=32. Data-parallel over batch:
core b processes image b. Full inputs in, full output out.

v3 design (per core):
  - x arrives pre-cast bf16 (host) as [128, 2, HW]; output leaves bf16
    [128, 2, HW]; host reorders/upcasts. Halves HBM traffic vs f32.
  - P1: QKV with stacked weights -> psum [96, 512]; DVE evacuates
    K@0/Q@32/V@64 of tA in one copy. tB holds a Q@0 replica (DMA).
  - Both row and col attention use FLIPPED applies (expe stationary,
    vts moving, 33-col output) -> per-stripe [pos, c] tiles land in
    outT_row [128w, H, 33] / outT_col [128h, W, 33] via cheap ACT copies.
    Both bounce to one DRAM tensor uc2 [(h w), 128pad]: row writes plain
    (sync/HWDGE), col writes ACCUMULATE (gpsimd/SWDGE cce add). The ones
    column of vts makes col 32 the softmax denominator Z.
  - uc2 is DMA-TRANSPOSED back in 8 chunks as u_tot^T [128c, (h w)]
    row-major; partition 32 carries Z (unused; Z handled on-chip).
  - Z: Z_row^T [w, h] view of outT_row is xbar-transposed to [h, w],
    added to the Z_col view of outT_col, reciprocal -> rscr -> broadcast
    rb loads (batched per tread chunk, issued from the ACT HWDGE queue).
  - P5: norm = u^T * rb (DVE); Wz matmuls; ACT evacuates psum to bf16;
    DVE adds residual at 2x; stores issued via gpsimd to keep the Sync
    queue free for transpose-reads.
  - Attention loops are software-pipelined: energies/exp/mask of batch k
    are emitted before the applies of batch k-1 so ACT/DVE latency hides
    behind the in-order PE stream.
"""
import sys

sys.path.insert(0, "/opt/trn_rl_repo")

import numpy as np
import ml_dtypes

import concourse.bass as bass
import concourse.mybir as mybir
from concourse import bacc, tile
from concourse.bass_utils import run_bass_kernel_spmd

B, C, H, W, CR = 8, 256, 128, 128, 32
HW = H * W
BF = ml_dtypes.bfloat16

_BUILD_CACHE = {}


def _build(with_qkv_bias: bool, with_z_bias: bool):
    nc = bacc.Bacc("TRN2", target_bir_lowering=False, debug=False, num_devices=8)
    dt = mybir.dt
    f32, bf16 = dt.float32, dt.bfloat16
    Exp = mybir.ActivationFunctionType.Exp
    ADD = mybir.AluOpType.add

    x_d = nc.dram_tensor("x", [128, 2, HW], bf16, kind="ExternalInput").ap()
    wkqvT_d = nc.dram_tensor("wkqvT", [C, 96], bf16, kind="ExternalInput").ap()
    wzT_d = nc.dram_tensor("wzT", [CR, C], bf16, kind="ExternalInput").ap()
    mask_d = nc.dram_tensor("mask1", [128, 128], bf16, kind="ExternalInput").ap()
    ident_d = nc.dram_tensor("identpad", [128, 32], bf16, kind="ExternalInput").ap()
    if with_qkv_bias:
        bvkq_d = nc.dram_tensor("bvkq", [1, 96], bf16, kind="ExternalInput").ap()
    if with_z_bias:
        bzr_d = nc.dram_tensor("bz_row", [1, C], bf16, kind="ExternalInput").ap()

    uc2_d = nc.dram_tensor("uc2", [HW, 128], bf16, kind="Internal").ap()
    rscr = nc.dram_tensor("rscr", [HW], bf16, kind="Internal").ap()
    out_d = nc.dram_tensor("out", [128, 2, HW], bf16, kind="ExternalOutput").ap()

    with tile.TileContext(nc) as tc:
        with (
            tc.tile_pool(name="persist", bufs=1) as pp,
            tc.tile_pool(name="work", bufs=2) as wp,
            tc.tile_pool(name="uqp", bufs=2) as uqp,
            tc.tile_pool(name="outw", bufs=3) as op,
            tc.tile_pool(name="rbp", bufs=2) as rbp,
            tc.tile_pool(name="rwork", bufs=2) as rp,
            tc.tile_pool(name="psBig", bufs=2, space="PSUM") as pBig,
            tc.tile_pool(name="psSmall", bufs=2, space="PSUM") as pSm,
            tc.tile_pool(name="psApl", bufs=2, space="PSUM") as pAp,
        ):
            # ---- persistent SBUF ----
            x_bf = pp.tile([128, 2, HW], bf16)
            # tA rows: K@0, Q@32, V@64. tB rows: Q@0.
            tA = pp.tile([128, H, W], bf16)
            tB = pp.tile([32, H, W], bf16)
            outT_row = pp.tile([128, H, 33], bf16)  # [w, h, c]
            outT_col = pp.tile([128, W, 33], bf16)  # [h, w, c]
            vts = pp.tile([128, W, 33], bf16)       # V^T stripes (+ones col)
            wkqvT = pp.tile([128, 2, 96], bf16)
            wzT = pp.tile([CR, C], bf16)
            mask1 = pp.tile([128, 128], bf16)
            ident = pp.tile([128, 32], bf16)
            zrow2 = pp.tile([128, 128], bf16)       # Z_row as [h, w]
            zsum = pp.tile([128, 128], f32)
            rC = pp.tile([128, 128], bf16)

            nc.sync.dma_start(out=wkqvT[:], in_=wkqvT_d.rearrange("(a p) m -> p a m", p=128))
            nc.sync.dma_start(out=wzT[:], in_=wzT_d)
            nc.sync.dma_start(out=mask1[:], in_=mask_d)
            nc.sync.dma_start(out=ident[:], in_=ident_d)
            if with_qkv_bias or with_z_bias:
                ones_row = pp.tile([1, 512], bf16)
                nc.vector.memset(ones_row[:], 1.0)
            if with_qkv_bias:
                bvkq = pp.tile([1, 96], bf16)
                nc.sync.dma_start(out=bvkq[:], in_=bvkq_d)
            if with_z_bias:
                bz_row = pp.tile([1, C], bf16)
                nc.sync.dma_start(out=bz_row[:], in_=bzr_d)

            nc.vector.memset(vts[:, :, 32:33], 1.0)

            # broadcast view of mask1 over the 8-stripe dim
            m = mask1[:]
            mask_b = bass.AP(tensor=m.tensor, offset=m.offset,
                             ap=[list(m.ap[0]), [0, 8], list(m.ap[1])])

            uc_row = uc2_d.rearrange("(h w) c -> w h c", w=128)  # [w, h, c]
            uc_col = uc2_d.rearrange("(h w) c -> h w c", w=128)  # [h, w, c]

            def transposes(g0, row_mode):
                pv = pSm.tile([128, 16, 32], f32, tag="sm")
                for j16 in range(16):
                    src = (tA[64:96, g0 + j16, :] if row_mode
                           else tA[64:96, :, g0 + j16])
                    nc.tensor.matmul(pv[:, j16, :], src, ident[64:96, :],
                                     start=True, stop=True)
                nc.vector.tensor_copy(vts[:, g0:g0 + 16, 0:32], pv[:])

            def energies(s0, row_mode):
                ps_e = pBig.tile([128, 8, 128], f32, tag="big")
                for j in range(8):
                    if row_mode:
                        lhsT, rhs = tA[0:32, s0 + j, :], tB[0:32, s0 + j, :]
                    else:
                        lhsT, rhs = tA[0:32, :, s0 + j], tB[0:32, :, s0 + j]
                    nc.tensor.matmul(ps_e[:, j, :], lhsT, rhs,
                                     start=True, stop=True)
                expe = wp.tile([128, 8, 128], bf16, tag="expe")
                nc.scalar.activation(expe[:], ps_e[:], Exp)
                if not row_mode:
                    nc.vector.tensor_mul(expe[:], expe[:], mask_b)
                return expe

            def warmers(n):
                for _ in range(n):
                    dps = pAp.tile([96, 512], f32, tag="ap")
                    nc.tensor.matmul(dps[:], wkqvT[:, 0, :],
                                     x_bf[:, 0, 0:512], start=True, stop=True)

            def applies(s0, row_mode, expe):
                psa = pAp.tile([128, 8, 33], f32, tag="ap")
                for j in range(8):
                    nc.tensor.matmul(psa[:, j, :], expe[:, j, :],
                                     vts[:, s0 + j, :], start=True, stop=True)
                if row_mode:
                    nc.vector.tensor_copy(outT_row[:, s0:s0 + 8, :], psa[:])
                    nc.sync.dma_start(out=uc_row[:, s0:s0 + 8, 0:33],
                                      in_=outT_row[:, s0:s0 + 8, :])
                else:
                    nc.vector.tensor_copy(outT_col[:, s0:s0 + 8, :], psa[:])
                    nc.gpsimd.dma_start(out=uc_col[:, s0:s0 + 8, 0:33],
                                        in_=outT_col[:, s0:s0 + 8, :],
                                        accum_op=ADD)

            # ================= Phase A: load + P1 + row attention ========
            pend = None  # (s0, row_mode, expe) pending apply
            for q in range(4):
                s = q * 4096
                nc.sync.dma_start(out=x_bf[:, :, s:s + 4096],
                                  in_=x_d[:, :, s:s + 4096])
                for chl in range(8):
                    ch = q * 8 + chl
                    px = ch * 512
                    h0 = ch * 4
                    ps = pSm.tile([96, 512], f32, tag="sm")
                    nc.tensor.matmul(ps[:], wkqvT[:, 0, :], x_bf[:, 0, px:px + 512],
                                     start=True, stop=False)
                    nc.tensor.matmul(ps[:], wkqvT[:, 1, :], x_bf[:, 1, px:px + 512],
                                     start=False, stop=not with_qkv_bias)
                    if with_qkv_bias:
                        nc.tensor.matmul(ps[:], bvkq[:], ones_row[:],
                                         start=False, stop=True)
                    ps3 = ps[:].rearrange("p (a b) -> p a b", b=128)
                    nc.vector.tensor_copy(tA[0:96, h0:h0 + 4, :], ps3)
                hq = q * 32
                nc.sync.dma_start(out=tB[0:32, hq:hq + 32, :],
                                  in_=tA[32:64, hq:hq + 32, :])   # Q@0
                for g in range(2):
                    g0 = hq + g * 16
                    transposes(g0, True)
                    for b8 in range(2):
                        s0 = g0 + b8 * 8
                        expe = energies(s0, True)
                        if pend is not None:
                            applies(*pend)
                        pend = (s0, True, expe)
            applies(*pend)
            pend = None

            # Z_row^T [w, h] -> [h, w] via xbar transpose
            zrowT_c = pp.tile([128, 128], bf16)
            nc.vector.tensor_copy(zrowT_c[:], outT_row[:, :, 32])
            nc.sync.dma_start_transpose(zrow2[:], zrowT_c[:])

            # ================= Phase B: col attention ====================
            for wb in range(16):
                s0 = wb * 8
                if wb % 2 == 0:
                    transposes(s0, False)
                expe = energies(s0, False)
                warmers(3)
                if pend is not None:
                    applies(*pend)
                pend = (s0, False, expe)
            applies(*pend)

            # ================= Z -> 1/Z ==================================
            nc.vector.tensor_add(zsum[:], zrow2[:], outT_col[:, :, 32])
            rsq = pp.tile([128, 128], f32)
            nc.vector.reciprocal(rsq[:], zsum[:])
            nc.vector.tensor_copy(rC[:], rsq[:])
            nc.sync.dma_start(out=rscr.rearrange("(p f) -> p f", p=128), in_=rC[:])

            # ================= Phase D: transpose-read + P5 ==============
            for k in range(8):
                uqt = uqp.tile([128, 2048], bf16, tag="uq")
                nc.sync.dma_start_transpose(uqt[:], uc2_d[2048 * k:2048 * (k + 1), :])
                rb4 = rbp.tile([32, 2048], bf16, tag="rb")
                src = rscr[2048 * k:2048 * (k + 1)]
                bcast = bass.AP(tensor=src.tensor, offset=src.offset,
                                ap=[[0, 32]] + list(src.ap))
                nc.scalar.dma_start(out=rb4[:], in_=bcast)
                for cc in range(4):
                    cch = k * 4 + cc
                    px = cch * 512
                    tmp = rp.tile([32, 512], bf16, tag="tmp")
                    nc.vector.tensor_mul(tmp[:], uqt[0:32, cc * 512:(cc + 1) * 512],
                                         rb4[:, cc * 512:(cc + 1) * 512])
                    of_ps = pBig.tile([128, 2, 512], f32, tag="big")
                    for half in range(2):
                        nc.tensor.matmul(of_ps[:, half, :],
                                         wzT[:, half * 128:(half + 1) * 128],
                                         tmp[:], start=True, stop=not with_z_bias)
                        if with_z_bias:
                            nc.tensor.matmul(
                                of_ps[:, half, :],
                                bz_row[:, half * 128:(half + 1) * 128],
                                ones_row[:], start=False, stop=True)
                    t2 = op.tile([128, 2, 512], bf16, tag="t2")
                    nc.scalar.copy(t2[:], of_ps[:])
                    warmers(2)
                    of = op.tile([128, 2, 512], bf16, tag="of")
                    nc.vector.tensor_add(of[:], t2[:], x_bf[:, :, px:px + 512])
                    nc.gpsimd.dma_start(out=out_d[:, :, px:px + 512], in_=of[:])
    nc.compile()
    return nc


def _host_prep(Wq, bq, Wk, bk, Wv, bv, Wz, bz):
    wkqvT = np.ascontiguousarray(
        np.concatenate([Wk, Wq, Wv], axis=0).T).astype(BF)          # (256, 96)
    wzT = np.ascontiguousarray(Wz.T).astype(BF)                      # (32, 256)
    bz_row = np.asarray(bz, np.float32).reshape(1, C).astype(BF)
    eye = np.eye(128, dtype=np.float32)
    mask1 = np.ascontiguousarray(1.0 - eye).astype(BF)
    identpad = np.vstack([np.eye(32, dtype=np.float32)] * 4).astype(BF)
    bvkq = np.concatenate([bk, bq, bv]).reshape(1, 96).astype(BF)
    return wkqvT, wzT, bz_row, mask1, identpad, bvkq


def _prep_x(xb):
    # (256, H, W) f32 -> (128, 2, HW) bf16
    return np.ascontiguousarray(
        xb.reshape(2, 128, HW).transpose(1, 0, 2)).astype(BF)


def kernel(x, Wq, bq, Wk, bk, Wv, bv, Wz, bz):
    x = np.asarray(x, np.float32)
    wkqvT, wzT, bz_row, mask1, identpad, bvkq = _host_prep(
        np.asarray(Wq, np.float32), np.asarray(bq, np.float32),
        np.asarray(Wk, np.float32), np.asarray(bk, np.float32),
        np.asarray(Wv, np.float32), np.asarray(bv, np.float32),
        np.asarray(Wz, np.float32), np.asarray(bz, np.float32))
    with_qkv_bias = bool(np.any(bvkq.astype(np.float32) != 0.0))
    with_z_bias = bool(np.any(bz_row.astype(np.float32) != 0.0))

    key = (with_qkv_bias, with_z_bias)
    if key not in _BUILD_CACHE:
        _BUILD_CACHE[key] = _build(*key)
    nc = _BUILD_CACHE[key]

    in_maps = []
    for b in range(B):
        m = dict(
            x=_prep_x(x[b]),
            wkqvT=wkqvT, wzT=wzT, mask1=mask1, identpad=identpad,
        )
        if with_qkv_bias:
            m["bvkq"] = bvkq
        if with_z_bias:
            m["bz_row"] = bz_row
        in_maps.append(m)

    res = run_bass_kernel_spmd(nc, in_maps, core_ids=list(range(8)))
    out = np.stack([
        np.asarray(res.results[b]["out"]).astype(np.float32)
        .transpose(1, 0, 2).reshape(C, H, W)
        for b in range(B)
    ])
    return out


# revision 13
# speedup vs baseline: 1.2111x; 1.2111x over previous
"""Criss-cross attention block (CCNet) Bass/Tile kernel for Trainium2.

Shapes (hardcoded): B=8, C=256, H=W=128, CR=32. Data-parallel over batch:
core b processes image b. Full inputs in, full output out.

v3 design (per core):
  - x arrives pre-cast bf16 (host) as [128, 2, HW]; output leaves bf16
    [128, 2, HW]; host reorders/upcasts. Halves HBM traffic vs f32.
  - P1: QKV with stacked weights -> psum [96, 512]; DVE evacuates
    K@0/Q@32/V@64 of tA in one copy. tB holds a Q@0 replica (DMA).
  - Both row and col attention use FLIPPED applies (expe stationary,
    vts moving, 33-col output) -> per-stripe [pos, c] tiles land in
    outT_row [128w, H, 33] / outT_col [128h, W, 33] via cheap ACT copies.
    Both bounce to one DRAM tensor uc2 [(h w), 128pad]: row writes plain
    (sync/HWDGE), col writes ACCUMULATE (gpsimd/SWDGE cce add). The ones
    column of vts makes col 32 the softmax denominator Z.
  - uc2 is DMA-TRANSPOSED back in 8 chunks as u_tot^T [128c, (h w)]
    row-major; partition 32 carries Z (unused; Z handled on-chip).
  - Z: Z_row^T [w, h] view of outT_row is xbar-transposed to [h, w],
    added to the Z_col view of outT_col, reciprocal -> rscr -> broadcast
    rb loads (batched per tread chunk, issued from the ACT HWDGE queue).
  - P5: norm = u^T * rb (DVE); Wz matmuls; ACT evacuates psum to bf16;
    DVE adds residual at 2x; stores issued via gpsimd to keep the Sync
    queue free for transpose-reads.
  - Attention loops are software-pipelined: energies/exp/mask of batch k
    are emitted before the applies of batch k-1 so ACT/DVE latency hides
    behind the in-order PE stream.
"""
import sys

sys.path.insert(0, "/opt/trn_rl_repo")

import numpy as np
import ml_dtypes

import concourse.bass as bass
import concourse.mybir as mybir
from concourse import bacc, tile
from concourse.bass_utils import run_bass_kernel_spmd

B, C, H, W, CR = 8, 256, 128, 128, 32
HW = H * W
BF = ml_dtypes.bfloat16

_BUILD_CACHE = {}


def _build(with_qkv_bias: bool, with_z_bias: bool):
    nc = bacc.Bacc("TRN2", target_bir_lowering=False, debug=False, num_devices=8)
    dt = mybir.dt
    f32, bf16 = dt.float32, dt.bfloat16
    Exp = mybir.ActivationFunctionType.Exp
    ADD = mybir.AluOpType.add

    x_d = nc.dram_tensor("x", [128, 2, HW], bf16, kind="ExternalInput").ap()
    wkqvT_d = nc.dram_tensor("wkqvT", [C, 96], bf16, kind="ExternalInput").ap()
    wzT_d = nc.dram_tensor("wzT", [CR, C], bf16, kind="ExternalInput").ap()
    mask_d = nc.dram_tensor("mask1", [128, 128], bf16, kind="ExternalInput").ap()
    ident_d = nc.dram_tensor("identpad", [128, 32], bf16, kind="ExternalInput").ap()
    if with_qkv_bias:
        bvkq_d = nc.dram_tensor("bvkq", [1, 96], bf16, kind="ExternalInput").ap()
    if with_z_bias:
        bzr_d = nc.dram_tensor("bz_row", [1, C], bf16, kind="ExternalInput").ap()

    uc2_d = nc.dram_tensor("uc2", [HW, 128], bf16, kind="Internal").ap()
    rscr = nc.dram_tensor("rscr", [HW], bf16, kind="Internal").ap()
    out_d = nc.dram_tensor("out", [128, 2, HW], bf16, kind="ExternalOutput").ap()

    with tile.TileContext(nc) as tc:
        with (
            tc.tile_pool(name="persist", bufs=1) as pp,
            tc.tile_pool(name="work", bufs=2) as wp,
            tc.tile_pool(name="uqp", bufs=2) as uqp,
            tc.tile_pool(name="outw", bufs=3) as op,
            tc.tile_pool(name="rbp", bufs=2) as rbp,
            tc.tile_pool(name="rwork", bufs=2) as rp,
            tc.tile_pool(name="psBig", bufs=2, space="PSUM") as pBig,
            tc.tile_pool(name="psSmall", bufs=2, space="PSUM") as pSm,
            tc.tile_pool(name="psApl", bufs=2, space="PSUM") as pAp,
        ):
            # ---- persistent SBUF ----
            x_bf = pp.tile([128, 2, HW], bf16)
            # tA rows: K@0, Q@32, V@64. tB rows: Q@0.
            tA = pp.tile([128, H, W], bf16)
            tB = pp.tile([32, H, W], bf16)
            outT_row = pp.tile([128, H, 33], bf16)  # [w, h, c]
            outT_col = pp.tile([128, W, 33], bf16)  # [h, w, c]
            vts = pp.tile([128, W, 33], bf16)       # V^T stripes (+ones col)
            wkqvT = pp.tile([128, 2, 96], bf16)
            wzT = pp.tile([CR, C], bf16)
            mask1 = pp.tile([128, 128], bf16)
            ident = pp.tile([128, 32], bf16)
            zrow2 = pp.tile([128, 128], bf16)       # Z_row as [h, w]
            zsum = pp.tile([128, 128], f32)
            rC = pp.tile([128, 128], bf16)

            nc.sync.dma_start(out=wkqvT[:], in_=wkqvT_d.rearrange("(a p) m -> p a m", p=128))
            nc.sync.dma_start(out=wzT[:], in_=wzT_d)
            nc.sync.dma_start(out=mask1[:], in_=mask_d)
            nc.sync.dma_start(out=ident[:], in_=ident_d)
            if with_qkv_bias or with_z_bias:
                ones_row = pp.tile([1, 512], bf16)
                nc.vector.memset(ones_row[:], 1.0)
            if with_qkv_bias:
                bvkq = pp.tile([1, 96], bf16)
                nc.sync.dma_start(out=bvkq[:], in_=bvkq_d)
            if with_z_bias:
                bz_row = pp.tile([1, C], bf16)
                nc.sync.dma_start(out=bz_row[:], in_=bzr_d)

            nc.vector.memset(vts[:, :, 32:33], 1.0)

            # broadcast view of mask1 over the 8-stripe dim
            m = mask1[:]
            mask_b = bass.AP(tensor=m.tensor, offset=m.offset,
                             ap=[list(m.ap[0]), [0, 8], list(m.ap[1])])

            uc_row = uc2_d.rearrange("(h w) c -> w h c", w=128)  # [w, h, c]
            uc_col = uc2_d.rearrange("(h w) c -> h w c", w=128)  # [h, w, c]

            def transposes(g0, row_mode):
                pv = pSm.tile([128, 16, 32], f32, tag="sm")
                for j16 in range(16):
                    src = (tA[64:96, g0 + j16, :] if row_mode
                           else tA[64:96, :, g0 + j16])
                    nc.tensor.matmul(pv[:, j16, :], src, ident[64:96, :],
                                     start=True, stop=True)
                nc.vector.tensor_copy(vts[:, g0:g0 + 16, 0:32], pv[:])

            def energies(s0, row_mode):
                ps_e = pBig.tile([128, 8, 128], f32, tag="big")
                for j in range(8):
                    if row_mode:
                        lhsT, rhs = tA[0:32, s0 + j, :], tB[0:32, s0 + j, :]
                    else:
                        lhsT, rhs = tA[0:32, :, s0 + j], tB[0:32, :, s0 + j]
                    nc.tensor.matmul(ps_e[:, j, :], lhsT, rhs,
                                     start=True, stop=True)
                expe = wp.tile([128, 8, 128], bf16, tag="expe")
                nc.scalar.activation(expe[:], ps_e[:], Exp)
                if not row_mode:
                    nc.vector.tensor_mul(expe[:], expe[:], mask_b)
                return expe

            def applies(s0, row_mode, expe):
                psa = pAp.tile([128, 8, 33], f32, tag="ap")
                for j in range(8):
                    nc.tensor.matmul(psa[:, j, :], expe[:, j, :],
                                     vts[:, s0 + j, :], start=True, stop=True)
                if row_mode:
                    nc.vector.tensor_copy(outT_row[:, s0:s0 + 8, :], psa[:])
                    nc.sync.dma_start(out=uc_row[:, s0:s0 + 8, 0:33],
                                      in_=outT_row[:, s0:s0 + 8, :])
                else:
                    nc.vector.tensor_copy(outT_col[:, s0:s0 + 8, :], psa[:])
                    nc.gpsimd.dma_start(out=uc_col[:, s0:s0 + 8, 0:33],
                                        in_=outT_col[:, s0:s0 + 8, :],
                                        accum_op=ADD)

            # ================= Phase A: load + P1 + row attention ========
            pend = None  # (s0, row_mode, expe) pending apply
            for q in range(4):
                s = q * 4096
                nc.sync.dma_start(out=x_bf[:, :, s:s + 4096],
                                  in_=x_d[:, :, s:s + 4096])
                for chl in range(8):
                    ch = q * 8 + chl
                    px = ch * 512
                    h0 = ch * 4
                    ps = pSm.tile([96, 512], f32, tag="sm")
                    nc.tensor.matmul(ps[:], wkqvT[:, 0, :], x_bf[:, 0, px:px + 512],
                                     start=True, stop=False)
                    nc.tensor.matmul(ps[:], wkqvT[:, 1, :], x_bf[:, 1, px:px + 512],
                                     start=False, stop=not with_qkv_bias)
                    if with_qkv_bias:
                        nc.tensor.matmul(ps[:], bvkq[:], ones_row[:],
                                         start=False, stop=True)
                    ps3 = ps[:].rearrange("p (a b) -> p a b", b=128)
                    nc.vector.tensor_copy(tA[0:96, h0:h0 + 4, :], ps3)
                hq = q * 32
                nc.sync.dma_start(out=tB[0:32, hq:hq + 32, :],
                                  in_=tA[32:64, hq:hq + 32, :])   # Q@0
                for g in range(2):
                    g0 = hq + g * 16
                    transposes(g0, True)
                    for b8 in range(2):
                        s0 = g0 + b8 * 8
                        expe = energies(s0, True)
                        if pend is not None:
                            applies(*pend)
                        pend = (s0, True, expe)
            applies(*pend)
            pend = None

            # Z_row^T [w, h] -> [h, w] via xbar transpose
            zrowT_c = pp.tile([128, 128], bf16)
            nc.vector.tensor_copy(zrowT_c[:], outT_row[:, :, 32])
            nc.sync.dma_start_transpose(zrow2[:], zrowT_c[:])

            # ================= Phase B: col attention ====================
            for wb in range(16):
                s0 = wb * 8
                if wb % 2 == 0:
                    transposes(s0, False)
                expe = energies(s0, False)
                if pend is not None:
                    applies(*pend)
                pend = (s0, False, expe)
            applies(*pend)

            # ================= Z -> 1/Z ==================================
            nc.vector.tensor_add(zsum[:], zrow2[:], outT_col[:, :, 32])
            rsq = pp.tile([128, 128], f32)
            nc.vector.reciprocal(rsq[:], zsum[:])
            nc.vector.tensor_copy(rC[:], rsq[:])
            nc.sync.dma_start(out=rscr.rearrange("(p f) -> p f", p=128), in_=rC[:])

            # ================= Phase D: transpose-read + P5 ==============
            for k in range(8):
                uqt = uqp.tile([128, 2048], bf16, tag="uq")
                nc.sync.dma_start_transpose(uqt[:], uc2_d[2048 * k:2048 * (k + 1), :])
                rb4 = rbp.tile([32, 2048], bf16, tag="rb")
                src = rscr[2048 * k:2048 * (k + 1)]
                bcast = bass.AP(tensor=src.tensor, offset=src.offset,
                                ap=[[0, 32]] + list(src.ap))
                nc.scalar.dma_start(out=rb4[:], in_=bcast)
                for cc in range(4):
                    cch = k * 4 + cc
                    px = cch * 512
                    tmp = rp.tile([32, 512], bf16, tag="tmp")
                    nc.vector.tensor_mul(tmp[:], uqt[0:32, cc * 512:(cc + 1) * 512],
                                         rb4[:, cc * 512:(cc + 1) * 512])
                    of_ps = pBig.tile([128, 2, 512], f32, tag="big")
                    for half in range(2):
                        nc.tensor.matmul(of_ps[:, half, :],
                                         wzT[:, half * 128:(half + 1) * 128],
                                         tmp[:], start=True, stop=not with_z_bias)
                        if with_z_bias:
                            nc.tensor.matmul(
                                of_ps[:, half, :],
                                bz_row[:, half * 128:(half + 1) * 128],
                                ones_row[:], start=False, stop=True)
                    t2 = op.tile([128, 2, 512], bf16, tag="t2")
                    nc.scalar.copy(t2[:], of_ps[:])
                    of = op.tile([128, 2, 512], bf16, tag="of")
                    nc.vector.tensor_add(of[:], t2[:], x_bf[:, :, px:px + 512])
                    nc.sync.dma_start(out=out_d[:, :, px:px + 512], in_=of[:])
    nc.compile()
    return nc


def _host_prep(Wq, bq, Wk, bk, Wv, bv, Wz, bz):
    wkqvT = np.ascontiguousarray(
        np.concatenate([Wk, Wq, Wv], axis=0).T).astype(BF)          # (256, 96)
    wzT = np.ascontiguousarray(Wz.T).astype(BF)                      # (32, 256)
    bz_row = np.asarray(bz, np.float32).reshape(1, C).astype(BF)
    eye = np.eye(128, dtype=np.float32)
    mask1 = np.ascontiguousarray(1.0 - eye).astype(BF)
    identpad = np.vstack([np.eye(32, dtype=np.float32)] * 4).astype(BF)
    bvkq = np.concatenate([bk, bq, bv]).reshape(1, 96).astype(BF)
    return wkqvT, wzT, bz_row, mask1, identpad, bvkq


def _prep_x(xb):
    # (256, H, W) f32 -> (128, 2, HW) bf16
    return np.ascontiguousarray(
        xb.reshape(2, 128, HW).transpose(1, 0, 2)).astype(BF)


def kernel(x, Wq, bq, Wk, bk, Wv, bv, Wz, bz):
    x = np.asarray(x, np.float32)
    wkqvT, wzT, bz_row, mask1, identpad, bvkq = _host_prep(
        np.asarray(Wq, np.float32), np.asarray(bq, np.float32),
        np.asarray(Wk, np.float32), np.asarray(bk, np.float32),
        np.asarray(Wv, np.float32), np.asarray(bv, np.float32),
        np.asarray(Wz, np.float32), np.asarray(bz, np.float32))
    with_qkv_bias = bool(np.any(bvkq.astype(np.float32) != 0.0))
    with_z_bias = bool(np.any(bz_row.astype(np.float32) != 0.0))

    key = (with_qkv_bias, with_z_bias)
    if key not in _BUILD_CACHE:
        _BUILD_CACHE[key] = _build(*key)
    nc = _BUILD_CACHE[key]

    in_maps = []
    for b in range(B):
        m = dict(
            x=_prep_x(x[b]),
            wkqvT=wkqvT, wzT=wzT, mask1=mask1, identpad=identpad,
        )
        if with_qkv_bias:
            m["bvkq"] = bvkq
        if with_z_bias:
            m["bz_row"] = bz_row
        in_maps.append(m)

    res = run_bass_kernel_spmd(nc, in_maps, core_ids=list(range(8)))
    out = np.stack([
        np.asarray(res.results[b]["out"]).astype(np.float32)
        .transpose(1, 0, 2).reshape(C, H, W)
        for b in range(B)
    ])
    return out
